# revision 1
# baseline (speedup 1.0000x reference)
import numpy as np
import jax
import jax.numpy as jnp
from functools import partial

# Evoformer block, distributed over 8 NeuronCores.
# Sharding per hint: L (residue) axis sharded 8 ways for the pair path
# (pair rows [L/8, L, c_z]); MSA path sharded over sequences (N/8 per core,
# attention is within-sequence so it is embarrassingly parallel there).
# Triangle-mul contracts the full k axis -> right operand is all-gathered.
# Triangular attention is row-parallel; the 'ending' mode needs the
# transposed pair, i.e. one all-to-all-like resharding (handled by XLA
# collectives emitted from shard_map).

C_M, C_Z, H, C_HID_OPM = 128, 64, 4, 32
N_SEQ, L = 64, 256
M = 8  # cores

_mesh = None


def _get_mesh():
    global _mesh
    if _mesh is None:
        devs = jax.devices()[:M]
        _mesh = jax.sharding.Mesh(np.array(devs), ("x",))
    return _mesh


def _ln(x, g, b, eps=1e-5):
    mu = jnp.mean(x, axis=-1, keepdims=True)
    var = jnp.mean((x - mu) ** 2, axis=-1, keepdims=True)
    return (x - mu) / jnp.sqrt(var + eps) * g + b


def _evoformer_sharded(msa_l, pair_rows, p):
    """Runs inside shard_map. msa_l: [N/8, L, c_m] local sequences.
    pair_rows: [L/8, L, c_z] local pair rows."""
    ax = "x"

    # ---- MSA row attention with pair bias (sequence-sharded) ----
    pp = p["msa_attn"]
    d = C_M // H
    m = _ln(msa_l, pp["ln_m"]["g"], pp["ln_m"]["b"])
    # pair bias: compute from local pair rows, then all-gather rows
    z_loc = _ln(pair_rows, pp["ln_z"]["g"], pp["ln_z"]["b"])
    bias_loc = z_loc @ pp["pair_bias"]["w"]  # [L/8, L, H]
    bias = jax.lax.all_gather(bias_loc, ax, axis=0, tiled=True)  # [L, L, H]
    bias = jnp.transpose(bias, (2, 0, 1))[None]  # [1,h,L,L]

    q = (m @ pp["q"]["w"]).reshape(*m.shape[:2], H, d)
    k = (m @ pp["k"]["w"]).reshape(*m.shape[:2], H, d)
    v = (m @ pp["v"]["w"]).reshape(*m.shape[:2], H, d)
    attn = jax.nn.softmax(
        jnp.einsum("bihd,bjhd->bhij", q, k) / d**0.5 + bias, axis=-1
    )
    o = jnp.einsum("bhij,bjhd->bihd", attn, v).reshape(*m.shape[:2], C_M)
    gate = jax.nn.sigmoid(m @ pp["gate"]["w"] + pp["gate"]["b"])
    msa_l = msa_l + gate * (o @ pp["out"]["w"] + pp["out"]["b"])

    # ---- MSA transition (local) ----
    pt = p["msa_trans"]
    t = _ln(msa_l, pt["ln"]["g"], pt["ln"]["b"])
    t = jax.nn.relu(t @ pt["l1"]["w"] + pt["l1"]["b"])
    msa_l = msa_l + (t @ pt["l2"]["w"] + pt["l2"]["b"])

    # ---- Outer product mean: pair rows update (row-sharded output) ----
    po = p["opm"]
    mm = _ln(msa_l, po["ln"]["g"], po["ln"]["b"])
    left = mm @ po["left"]["w"] + po["left"]["b"]    # [N/8, L, c_hid]
    right = mm @ po["right"]["w"] + po["right"]["b"]  # [N/8, L, c_hid]
    # row block for this core: rows r0:r0+L/8 of the pair
    idx = jax.lax.axis_index(ax)
    r0 = idx * (L // M)
    left_rows = jax.lax.dynamic_slice_in_dim(left, r0, L // M, axis=1)
    # contract over ALL sequences: psum partial outer products
    outer = jnp.einsum("sic,sjd->ijcd", left_rows, right)  # local partial
    outer = jax.lax.psum(outer, ax) / N_SEQ
    Lr = outer.shape[0]
    pair_rows = pair_rows + (
        outer.reshape(Lr, L, C_HID_OPM * C_HID_OPM) @ po["out"]["w"]
        + po["out"]["b"]
    )

    # ---- Triangle multiplicative updates (row-sharded, gather k axis) ----
    for key, mode in (("tri_out", "outgoing"), ("tri_in", "incoming")):
        pq = p[key]
        z = _ln(pair_rows, pq["ln"]["g"], pq["ln"]["b"])
        lp = z @ pq["lp"]["w"] + pq["lp"]["b"]
        lg = jax.nn.sigmoid(z @ pq["lg"]["w"] + pq["lg"]["b"])
        rp = z @ pq["rp"]["w"] + pq["rp"]["b"]
        rg = jax.nn.sigmoid(z @ pq["rg"]["w"] + pq["rg"]["b"])
        left = lp * lg    # [L/8, L, c_z]
        right = rp * rg   # [L/8, L, c_z]
        if mode == "outgoing":
            # out[i,j,c] = sum_k left[i,k,c] right[j,k,c]; need all rows of right
            right_full = jax.lax.all_gather(right, ax, axis=0, tiled=True)
            out = jnp.einsum("ikc,jkc->ijc", left, right_full)
        else:
            # out[i,j,c] = sum_k left[k,i,c] right[k,j,c]; contract sharded k
            # columns i of local out block: need full i,j; psum partials
            out = jnp.einsum("kic,kjc->ijc", left, right)  # [L, L, c_z] partial
            out = jax.lax.psum(out, ax)
            out = jax.lax.dynamic_slice_in_dim(out, r0, L // M, axis=0)
        o = _ln(out, pq["ln_f"]["g"], pq["ln_f"]["b"]) @ pq["op"]["w"] + pq["op"]["b"]
        og = jax.nn.sigmoid(pair_rows @ pq["og"]["w"] + pq["og"]["b"])
        pair_rows = pair_rows + og * o

    # ---- Triangular attention, starting (row-parallel) ----
    def tri_attn_rows(pair_rows, pq):
        dz = C_Z // H
        z = _ln(pair_rows, pq["ln"]["g"], pq["ln"]["b"])
        Lr = z.shape[0]
        q = (z @ pq["q"]["w"]).reshape(Lr, L, H, dz)
        k = (z @ pq["k"]["w"]).reshape(Lr, L, H, dz)
        v = (z @ pq["v"]["w"]).reshape(Lr, L, H, dz)
        b_loc = z @ pq["bias"]["w"]  # [L/8, L, H]
        b_full = jax.lax.all_gather(b_loc, ax, axis=0, tiled=True)  # [L, L, H]
        b_full = jnp.transpose(b_full, (2, 0, 1))[:, None]  # [h,1,L,L]
        attn = jax.nn.softmax(
            jnp.einsum("ijhd,ikhd->hijk", q, k) / dz**0.5 + b_full, axis=-1
        )
        o = jnp.einsum("hijk,ikhd->ijhd", attn, v).reshape(Lr, L, C_Z)
        gate = jax.nn.sigmoid(pair_rows @ pq["gate"]["w"] + pq["gate"]["b"])
        return pair_rows + gate * (o @ pq["out"]["w"] + pq["out"]["b"])

    pair_rows = tri_attn_rows(pair_rows, p["tri_attn_start"])

    # ---- ending mode: transpose pair (all-to-all), run rows, transpose back
    pair_full = jax.lax.all_gather(pair_rows, ax, axis=0, tiled=True)
    pair_t_rows = jax.lax.dynamic_slice_in_dim(
        jnp.swapaxes(pair_full, 0, 1), r0, L // M, axis=0
    )
    pair_t_rows = tri_attn_rows(pair_t_rows, p["tri_attn_end"])

    # ---- pair transition (elementwise over rows of pair^T, fine) ----
    pt = p["pair_trans"]
    t = _ln(pair_t_rows, pt["ln"]["g"], pt["ln"]["b"])
    t = jax.nn.relu(t @ pt["l1"]["w"] + pt["l1"]["b"])
    pair_t_rows = pair_t_rows + (t @ pt["l2"]["w"] + pt["l2"]["b"])

    # return msa rows and TRANSPOSED pair rows (host undoes the transpose)
    return msa_l, pair_t_rows


def kernel(msa, pair, params):
    mesh = _get_mesh()
    from jax.sharding import PartitionSpec as P
    from jax.experimental.shard_map import shard_map

    params = jax.tree_util.tree_map(lambda a: jnp.asarray(np.asarray(a)), params)
    fn = shard_map(
        partial(_evoformer_sharded),
        mesh=mesh,
        in_specs=(P("x"), P("x"), None),
        out_specs=(P("x"), P("x")),
        check_rep=False,
    )
    fn = jax.jit(fn)
    msa_out, pair_t = fn(jnp.asarray(np.asarray(msa)), jnp.asarray(np.asarray(pair)), params)
    msa_out = np.asarray(jax.device_get(msa_out))
    pair_out = np.asarray(jax.device_get(pair_t)).swapaxes(0, 1)
    return msa_out.astype(np.float32), pair_out.astype(np.float32)


# revision 2
# speedup vs baseline: 118.2468x; 118.2468x over previous
import numpy as np
import jax
import jax.numpy as jnp
from functools import partial

# Evoformer block, distributed over 8 NeuronCores.
# Sharding per hint: L (residue) axis sharded 8 ways for the pair path
# (pair rows [L/8, L, c_z]); MSA path sharded over sequences (N/8 per core,
# attention is within-sequence so it is embarrassingly parallel there).
# Triangle-mul contracts the full k axis -> right operand is all-gathered.
# Triangular attention is row-parallel; the 'ending' mode needs the
# transposed pair, i.e. one all-to-all-like resharding (handled by XLA
# collectives emitted from shard_map).

C_M, C_Z, H, C_HID_OPM = 128, 64, 4, 32
N_SEQ, L = 64, 256
M = 8  # cores

_mesh = None


def _get_mesh():
    global _mesh
    if _mesh is None:
        devs = jax.devices()[:M]
        _mesh = jax.sharding.Mesh(np.array(devs), ("x",))
    return _mesh


def _ln(x, g, b, eps=1e-5):
    mu = jnp.mean(x, axis=-1, keepdims=True)
    var = jnp.mean((x - mu) ** 2, axis=-1, keepdims=True)
    return (x - mu) / jnp.sqrt(var + eps) * g + b


def _evoformer_sharded(msa_l, pair_rows, p):
    """Runs inside shard_map. msa_l: [N/8, L, c_m] local sequences.
    pair_rows: [L/8, L, c_z] local pair rows."""
    ax = "x"

    # ---- MSA row attention with pair bias (sequence-sharded) ----
    pp = p["msa_attn"]
    d = C_M // H
    m = _ln(msa_l, pp["ln_m"]["g"], pp["ln_m"]["b"])
    # pair bias: compute from local pair rows, then all-gather rows
    z_loc = _ln(pair_rows, pp["ln_z"]["g"], pp["ln_z"]["b"])
    bias_loc = z_loc @ pp["pair_bias"]["w"]  # [L/8, L, H]
    bias = jax.lax.all_gather(bias_loc, ax, axis=0, tiled=True)  # [L, L, H]
    bias = jnp.transpose(bias, (2, 0, 1))[None]  # [1,h,L,L]

    q = (m @ pp["q"]["w"]).reshape(*m.shape[:2], H, d)
    k = (m @ pp["k"]["w"]).reshape(*m.shape[:2], H, d)
    v = (m @ pp["v"]["w"]).reshape(*m.shape[:2], H, d)
    attn = jax.nn.softmax(
        jnp.einsum("bihd,bjhd->bhij", q, k) / d**0.5 + bias, axis=-1
    )
    o = jnp.einsum("bhij,bjhd->bihd", attn, v).reshape(*m.shape[:2], C_M)
    gate = jax.nn.sigmoid(m @ pp["gate"]["w"] + pp["gate"]["b"])
    msa_l = msa_l + gate * (o @ pp["out"]["w"] + pp["out"]["b"])

    # ---- MSA transition (local) ----
    pt = p["msa_trans"]
    t = _ln(msa_l, pt["ln"]["g"], pt["ln"]["b"])
    t = jax.nn.relu(t @ pt["l1"]["w"] + pt["l1"]["b"])
    msa_l = msa_l + (t @ pt["l2"]["w"] + pt["l2"]["b"])

    # ---- Outer product mean: pair rows update (row-sharded output) ----
    po = p["opm"]
    mm = _ln(msa_l, po["ln"]["g"], po["ln"]["b"])
    left = mm @ po["left"]["w"] + po["left"]["b"]    # [N/8, L, c_hid]
    right = mm @ po["right"]["w"] + po["right"]["b"]  # [N/8, L, c_hid]
    # row block for this core: rows r0:r0+L/8 of the pair
    idx = jax.lax.axis_index(ax)
    r0 = idx * (L // M)
    left_rows = jax.lax.dynamic_slice_in_dim(left, r0, L // M, axis=1)
    # contract over ALL sequences: psum partial outer products
    outer = jnp.einsum("sic,sjd->ijcd", left_rows, right)  # local partial
    outer = jax.lax.psum(outer, ax) / N_SEQ
    Lr = outer.shape[0]
    pair_rows = pair_rows + (
        outer.reshape(Lr, L, C_HID_OPM * C_HID_OPM) @ po["out"]["w"]
        + po["out"]["b"]
    )

    # ---- Triangle multiplicative updates (row-sharded, gather k axis) ----
    for key, mode in (("tri_out", "outgoing"), ("tri_in", "incoming")):
        pq = p[key]
        z = _ln(pair_rows, pq["ln"]["g"], pq["ln"]["b"])
        lp = z @ pq["lp"]["w"] + pq["lp"]["b"]
        lg = jax.nn.sigmoid(z @ pq["lg"]["w"] + pq["lg"]["b"])
        rp = z @ pq["rp"]["w"] + pq["rp"]["b"]
        rg = jax.nn.sigmoid(z @ pq["rg"]["w"] + pq["rg"]["b"])
        left = lp * lg    # [L/8, L, c_z]
        right = rp * rg   # [L/8, L, c_z]
        if mode == "outgoing":
            # out[i,j,c] = sum_k left[i,k,c] right[j,k,c]; need all rows of right
            right_full = jax.lax.all_gather(right, ax, axis=0, tiled=True)
            out = jnp.einsum("ikc,jkc->ijc", left, right_full)
        else:
            # out[i,j,c] = sum_k left[k,i,c] right[k,j,c]; contract sharded k
            # columns i of local out block: need full i,j; psum partials
            out = jnp.einsum("kic,kjc->ijc", left, right)  # [L, L, c_z] partial
            out = jax.lax.psum(out, ax)
            out = jax.lax.dynamic_slice_in_dim(out, r0, L // M, axis=0)
        o = _ln(out, pq["ln_f"]["g"], pq["ln_f"]["b"]) @ pq["op"]["w"] + pq["op"]["b"]
        og = jax.nn.sigmoid(pair_rows @ pq["og"]["w"] + pq["og"]["b"])
        pair_rows = pair_rows + og * o

    # ---- Triangular attention, starting (row-parallel) ----
    def tri_attn_rows(pair_rows, pq):
        dz = C_Z // H
        z = _ln(pair_rows, pq["ln"]["g"], pq["ln"]["b"])
        Lr = z.shape[0]
        q = (z @ pq["q"]["w"]).reshape(Lr, L, H, dz)
        k = (z @ pq["k"]["w"]).reshape(Lr, L, H, dz)
        v = (z @ pq["v"]["w"]).reshape(Lr, L, H, dz)
        b_loc = z @ pq["bias"]["w"]  # [L/8, L, H]
        b_full = jax.lax.all_gather(b_loc, ax, axis=0, tiled=True)  # [L, L, H]
        b_full = jnp.transpose(b_full, (2, 0, 1))[:, None]  # [h,1,L,L]
        attn = jax.nn.softmax(
            jnp.einsum("ijhd,ikhd->hijk", q, k) / dz**0.5 + b_full, axis=-1
        )
        o = jnp.einsum("hijk,ikhd->ijhd", attn, v).reshape(Lr, L, C_Z)
        gate = jax.nn.sigmoid(pair_rows @ pq["gate"]["w"] + pq["gate"]["b"])
        return pair_rows + gate * (o @ pq["out"]["w"] + pq["out"]["b"])

    pair_rows = tri_attn_rows(pair_rows, p["tri_attn_start"])

    # ---- ending mode: transpose pair (all-to-all), run rows, transpose back
    pair_full = jax.lax.all_gather(pair_rows, ax, axis=0, tiled=True)
    pair_t_rows = jax.lax.dynamic_slice_in_dim(
        jnp.swapaxes(pair_full, 0, 1), r0, L // M, axis=0
    )
    pair_t_rows = tri_attn_rows(pair_t_rows, p["tri_attn_end"])

    # ---- pair transition (elementwise over rows of pair^T, fine) ----
    pt = p["pair_trans"]
    t = _ln(pair_t_rows, pt["ln"]["g"], pt["ln"]["b"])
    t = jax.nn.relu(t @ pt["l1"]["w"] + pt["l1"]["b"])
    pair_t_rows = pair_t_rows + (t @ pt["l2"]["w"] + pt["l2"]["b"])

    # return msa rows and TRANSPOSED pair rows (host undoes the transpose)
    return msa_l, pair_t_rows


_cache = {}


def _get_fn():
    if "fn" not in _cache:
        mesh = _get_mesh()
        from jax.sharding import PartitionSpec as P
        from jax.experimental.shard_map import shard_map

        fn = shard_map(
            _evoformer_sharded,
            mesh=mesh,
            in_specs=(P("x"), P("x"), None),
            out_specs=(P("x"), P("x")),
            check_rep=False,
        )
        _cache["fn"] = jax.jit(fn)
    return _cache["fn"]


def _device_params(params):
    key = id(params)
    if _cache.get("pkey") != key:
        _cache["pdev"] = jax.tree_util.tree_map(
            lambda a: jnp.asarray(np.asarray(a)), params
        )
        _cache["pkey"] = key
    return _cache["pdev"]


def kernel(msa, pair, params):
    fn = _get_fn()
    p = _device_params(params)
    msa_out, pair_t = fn(
        jnp.asarray(np.asarray(msa)), jnp.asarray(np.asarray(pair)), p
    )
    msa_out = np.asarray(jax.device_get(msa_out))
    pair_out = np.asarray(jax.device_get(pair_t)).swapaxes(0, 1)
    return msa_out.astype(np.float32), pair_out.astype(np.float32)


# revision 4
# speedup vs baseline: 143.4150x; 1.2128x over previous
import numpy as np
import jax
import jax.numpy as jnp
from functools import partial

# Evoformer block, distributed over 8 NeuronCores.
# Sharding per hint: L (residue) axis sharded 8 ways for the pair path
# (pair rows [L/8, L, c_z]); MSA path sharded over sequences (N/8 per core,
# attention is within-sequence so it is embarrassingly parallel there).
# Triangle-mul contracts the full k axis -> right operand is all-gathered.
# Triangular attention is row-parallel; the 'ending' mode needs the
# transposed pair, i.e. one all-to-all-like resharding (handled by XLA
# collectives emitted from shard_map).

C_M, C_Z, H, C_HID_OPM = 128, 64, 4, 32
N_SEQ, L = 64, 256
M = 8  # cores

_mesh = None


def _get_mesh():
    global _mesh
    if _mesh is None:
        devs = jax.devices()[:M]
        _mesh = jax.sharding.Mesh(np.array(devs), ("x",))
    return _mesh


def _ln(x, g, b, eps=1e-5):
    mu = jnp.mean(x, axis=-1, keepdims=True)
    var = jnp.mean((x - mu) ** 2, axis=-1, keepdims=True)
    return (x - mu) / jnp.sqrt(var + eps) * g + b


def _evoformer_sharded(msa_l, pair_rows, p):
    """Runs inside shard_map. msa_l: [N/8, L, c_m] local sequences.
    pair_rows: [L/8, L, c_z] local pair rows."""
    ax = "x"

    # ---- MSA row attention with pair bias (sequence-sharded) ----
    pp = p["msa_attn"]
    d = C_M // H
    m = _ln(msa_l, pp["ln_m"]["g"], pp["ln_m"]["b"])
    # pair bias: compute from local pair rows, then all-gather rows
    z_loc = _ln(pair_rows, pp["ln_z"]["g"], pp["ln_z"]["b"])
    bias_loc = z_loc @ pp["pair_bias"]["w"]  # [L/8, L, H]
    bias = jax.lax.all_gather(bias_loc, ax, axis=0, tiled=True)  # [L, L, H]
    bias = jnp.transpose(bias, (2, 0, 1))[None]  # [1,h,L,L]

    q = (m @ pp["q"]["w"]).reshape(*m.shape[:2], H, d)
    k = (m @ pp["k"]["w"]).reshape(*m.shape[:2], H, d)
    v = (m @ pp["v"]["w"]).reshape(*m.shape[:2], H, d)
    attn = jax.nn.softmax(
        jnp.einsum("bihd,bjhd->bhij", q, k) / d**0.5 + bias, axis=-1
    )
    o = jnp.einsum("bhij,bjhd->bihd", attn, v).reshape(*m.shape[:2], C_M)
    gate = jax.nn.sigmoid(m @ pp["gate"]["w"] + pp["gate"]["b"])
    msa_l = msa_l + gate * (o @ pp["out"]["w"] + pp["out"]["b"])

    # ---- MSA transition (local) ----
    pt = p["msa_trans"]
    t = _ln(msa_l, pt["ln"]["g"], pt["ln"]["b"])
    t = jax.nn.relu(t @ pt["l1"]["w"] + pt["l1"]["b"])
    msa_l = msa_l + (t @ pt["l2"]["w"] + pt["l2"]["b"])

    # ---- Outer product mean: pair rows update (row-sharded output) ----
    po = p["opm"]
    mm = _ln(msa_l, po["ln"]["g"], po["ln"]["b"])
    left = mm @ po["left"]["w"] + po["left"]["b"]    # [N/8, L, c_hid]
    right = mm @ po["right"]["w"] + po["right"]["b"]  # [N/8, L, c_hid]
    # row block for this core: rows r0:r0+L/8 of the pair
    idx = jax.lax.axis_index(ax)
    r0 = idx * (L // M)
    # all-gather the (small) left/right projections over sequences, then
    # contract all 64 sequences locally for this core's row block — cheaper
    # than psum-ing an 8.4MB partial outer product.
    left_f = jax.lax.all_gather(left, ax, axis=0, tiled=True)    # [N, L, c]
    right_f = jax.lax.all_gather(right, ax, axis=0, tiled=True)  # [N, L, c]
    left_rows = jax.lax.dynamic_slice_in_dim(left_f, r0, L // M, axis=1)
    outer = jnp.einsum("sic,sjd->ijcd", left_rows, right_f) / N_SEQ
    Lr = outer.shape[0]
    pair_rows = pair_rows + (
        outer.reshape(Lr, L, C_HID_OPM * C_HID_OPM) @ po["out"]["w"]
        + po["out"]["b"]
    )

    # ---- Triangle multiplicative updates (row-sharded, gather k axis) ----
    for key, mode in (("tri_out", "outgoing"), ("tri_in", "incoming")):
        pq = p[key]
        z = _ln(pair_rows, pq["ln"]["g"], pq["ln"]["b"])
        lp = z @ pq["lp"]["w"] + pq["lp"]["b"]
        lg = jax.nn.sigmoid(z @ pq["lg"]["w"] + pq["lg"]["b"])
        rp = z @ pq["rp"]["w"] + pq["rp"]["b"]
        rg = jax.nn.sigmoid(z @ pq["rg"]["w"] + pq["rg"]["b"])
        left = lp * lg    # [L/8, L, c_z]
        right = rp * rg   # [L/8, L, c_z]
        if mode == "outgoing":
            # out[i,j,c] = sum_k left[i,k,c] right[j,k,c]; need all rows of right
            right_full = jax.lax.all_gather(right, ax, axis=0, tiled=True)
            out = jnp.einsum("ikc,jkc->ijc", left, right_full)
        else:
            # out[i,j,c] = sum_k left[k,i,c] right[k,j,c]; gather the k axis
            # and compute only this core's i rows (8x less einsum than the
            # full-partial + psum formulation, and AG < AR on the wire).
            left_full = jax.lax.all_gather(left, ax, axis=0, tiled=True)
            right_full = jax.lax.all_gather(right, ax, axis=0, tiled=True)
            left_cols = jax.lax.dynamic_slice_in_dim(
                left_full, r0, L // M, axis=1
            )  # [L, L/8, c]
            out = jnp.einsum("kic,kjc->ijc", left_cols, right_full)
        o = _ln(out, pq["ln_f"]["g"], pq["ln_f"]["b"]) @ pq["op"]["w"] + pq["op"]["b"]
        og = jax.nn.sigmoid(pair_rows @ pq["og"]["w"] + pq["og"]["b"])
        pair_rows = pair_rows + og * o

    # ---- Triangular attention, starting (row-parallel) ----
    def tri_attn_rows(pair_rows, pq):
        dz = C_Z // H
        z = _ln(pair_rows, pq["ln"]["g"], pq["ln"]["b"])
        Lr = z.shape[0]
        q = (z @ pq["q"]["w"]).reshape(Lr, L, H, dz)
        k = (z @ pq["k"]["w"]).reshape(Lr, L, H, dz)
        v = (z @ pq["v"]["w"]).reshape(Lr, L, H, dz)
        b_loc = z @ pq["bias"]["w"]  # [L/8, L, H]
        b_full = jax.lax.all_gather(b_loc, ax, axis=0, tiled=True)  # [L, L, H]
        b_full = jnp.transpose(b_full, (2, 0, 1))[:, None]  # [h,1,L,L]
        attn = jax.nn.softmax(
            jnp.einsum("ijhd,ikhd->hijk", q, k) / dz**0.5 + b_full, axis=-1
        )
        o = jnp.einsum("hijk,ikhd->ijhd", attn, v).reshape(Lr, L, C_Z)
        gate = jax.nn.sigmoid(pair_rows @ pq["gate"]["w"] + pq["gate"]["b"])
        return pair_rows + gate * (o @ pq["out"]["w"] + pq["out"]["b"])

    pair_rows = tri_attn_rows(pair_rows, p["tri_attn_start"])

    # ---- ending mode: transpose pair (all-to-all), run rows, transpose back
    pair_full = jax.lax.all_gather(pair_rows, ax, axis=0, tiled=True)
    pair_t_rows = jax.lax.dynamic_slice_in_dim(
        jnp.swapaxes(pair_full, 0, 1), r0, L // M, axis=0
    )
    pair_t_rows = tri_attn_rows(pair_t_rows, p["tri_attn_end"])

    # ---- pair transition (elementwise over rows of pair^T, fine) ----
    pt = p["pair_trans"]
    t = _ln(pair_t_rows, pt["ln"]["g"], pt["ln"]["b"])
    t = jax.nn.relu(t @ pt["l1"]["w"] + pt["l1"]["b"])
    pair_t_rows = pair_t_rows + (t @ pt["l2"]["w"] + pt["l2"]["b"])

    # return msa rows and TRANSPOSED pair rows (host undoes the transpose)
    return msa_l, pair_t_rows


_cache = {}


def _get_fn():
    if "fn" not in _cache:
        mesh = _get_mesh()
        from jax.sharding import PartitionSpec as P
        from jax.experimental.shard_map import shard_map

        fn = shard_map(
            _evoformer_sharded,
            mesh=mesh,
            in_specs=(P("x"), P("x"), None),
            out_specs=(P("x"), P("x")),
            check_rep=False,
        )
        _cache["fn"] = jax.jit(fn)
    return _cache["fn"]


def _device_params(params):
    key = id(params)
    if _cache.get("pkey") != key:
        _cache["pdev"] = jax.tree_util.tree_map(
            lambda a: jnp.asarray(np.asarray(a)), params
        )
        _cache["pkey"] = key
    return _cache["pdev"]


def kernel(msa, pair, params):
    fn = _get_fn()
    p = _device_params(params)
    msa_out, pair_t = fn(
        jnp.asarray(np.asarray(msa)), jnp.asarray(np.asarray(pair)), p
    )
    msa_out = np.asarray(jax.device_get(msa_out))
    pair_out = np.asarray(jax.device_get(pair_t)).swapaxes(0, 1)
    return msa_out.astype(np.float32), pair_out.astype(np.float32)
